# revision 1
# baseline (speedup 1.0000x reference)
"""Multi-head self-attention (B=2, L=2048, D=1024, H=16, causal) on 8
Trainium2 NeuronCores.

Sharding: tensor-parallel over heads x data-parallel over batch.
Core c (0..7) handles batch b = c//4 and heads 4*(c%4) .. 4*(c%4)+3.
Each core computes partial = (softmax(qk^T/8) @ v_heads) @ Wo[:, cols]^T of
shape [L, D]; the host sums the 4 partials of each batch group.

Per-core kernel (all matmuls in fp32r = full-rate TF32-like):
  - host supplies x^T so q^T,k^T [256,L] and v [L,256] come straight from
    PE matmuls (no on-device transposes anywhere)
  - scores are computed TRANSPOSED (S^T = k q^T per 128-row key chunk,
    causal tiles only); exp runs on ScalarE directly PSUM->SBUF producing
    P^T in exactly the layout the PV matmul consumes; the 1/sqrt(dh) scale
    and the causal mask of the diagonal block (additive -1e5) are folded in
  - softmax denominators come free as a ones-column appended to v; they are
    broadcast across partitions with a K=1 ones matmul, inverted with a
    fast-reciprocal, and the normalize multiply is fused into the PSUM
    evacuation of the attention output
  - attention output is produced transposed ([64,L] per head), which is
    exactly the lhsT the output projection needs
"""

import numpy as np

B, L, D, H = 2, 2048, 1024, 16
DH = D // H  # 64
HPC = H // 2 // 4  # unused sanity
N_CORES = 8
HEADS_PER_CORE = 4
HD = HEADS_PER_CORE * DH  # 256 head dims per core
NK = D // 128  # 8 contraction chunks
LT = L // 128  # 16 L tiles
NG = L // 512  # 4 column groups

_CACHE = {}


# ---------------------------------------------------------------------------
# walrus compat: this compiler build accepts at most ONE sync-wait command
# per instruction, while TileContext attaches one wait per producer proc.
# Hoist surplus waits onto same-engine NOPs inserted just before the
# offending instruction (identical AND semantics).
# ---------------------------------------------------------------------------
def _split_waits(nc):
    import bass_rust
    import concourse.mybir as mybir

    for fn in nc.m.functions:
        for bb in fn.blocks:
            insts = list(bb.instructions)
            out = []
            changed = False
            for inst in insts:
                si = inst.sync_info
                waits = list(si.on_wait) if si is not None and si.on_wait else []
                if len(waits) > 1:
                    changed = True
                    for w in waits[:-1]:
                        out.append(
                            mybir.InstNoOp(
                                name=nc.get_next_instruction_name(),
                                engine=inst.engine,
                                bass_nofuse=True,
                                sync_info=bass_rust.SyncInfo(
                                    on_wait=[w], on_update=[]
                                ),
                            )
                        )
                    inst.sync_info = bass_rust.SyncInfo(
                        on_wait=[waits[-1]], on_update=list(si.on_update or [])
                    )
                out.append(inst)
            if changed:
                try:
                    bb.instructions = out
                except Exception:
                    bb.instructions.clear()
                    bb.instructions.extend(out)


def _act_reciprocal(nc, mybir, out_ap, in_ap):
    """ScalarE Reciprocal via direct InstActivation construction (the bass
    wrapper refuses it; accuracy here is ~1e-5 rel which is far below the
    fp32r operand rounding of this kernel, and the softmax denominators are
    strictly positive and well-scaled)."""
    AF = mybir.ActivationFunctionType
    eng = nc.scalar
    f32 = mybir.dt.float32
    ins = [
        eng.lower_ap(in_ap),
        eng.lower_ap(nc.const_aps.scalar_like(0.0, in_ap)),
        mybir.ImmediateValue(dtype=f32, value=1.0),
        mybir.ImmediateValue(dtype=f32, value=0.0),
    ]
    return eng.add_instruction(
        mybir.InstActivation(
            name=nc.get_next_instruction_name(),
            func=AF.Reciprocal,
            ins=ins,
            outs=[eng.lower_ap(out_ap)],
        )
    )


def _build_program():
    import concourse.bass as bass
    import concourse.mybir as mybir
    import concourse.tile as tile

    f32 = mybir.dt.float32
    f32r = mybir.dt.float32r
    AF = mybir.ActivationFunctionType

    nc = bass.Bass("TRN2", target_bir_lowering=False, debug=False)
    xT_d = nc.dram_tensor("xT", [D, L], f32, kind="ExternalInput")
    wq_d = nc.dram_tensor("wqT", [D, HD], f32, kind="ExternalInput")
    wk_d = nc.dram_tensor("wkT", [D, HD], f32, kind="ExternalInput")
    wv_d = nc.dram_tensor("wvT", [D, HD], f32, kind="ExternalInput")
    wo_d = nc.dram_tensor("woT", [HD, D], f32, kind="ExternalInput")
    tm_d = nc.dram_tensor("trimask", [128, 128], f32, kind="ExternalInput")
    out_d = nc.dram_tensor("out", [L, D], f32, kind="ExternalOutput")

    with tile.TileContext(nc, pool_alloc_mode="queue") as tc:
        with tc.tile_pool(name="persist", bufs=1) as persist:
            qTr = persist.tile([128, 2, L], f32r)
            kTr = persist.tile([128, 2, L], f32r)
            v_sb = persist.tile([128, LT, HEADS_PER_CORE * (DH + 1)], f32r)
            ones_l = persist.tile([1, 128], f32r)
            tm_t = persist.tile([128, 128], f32)
            woTr = persist.tile([128, 2, D], f32r)

            nc.sync.dma_start(tm_t[:], tm_d[:])

            # ---------------- phase A: projections ----------------
            with (
                tc.tile_pool(name="xtr", bufs=1) as xtrp,
                tc.tile_pool(name="wr", bufs=1) as wrp,
                tc.tile_pool(name="lda", bufs=3) as lda,
                tc.tile_pool(name="psA", bufs=8, space="PSUM") as psA,
            ):
                xTr = [xtrp.tile([128, L], f32r, name=f"xTr{c}") for c in range(NK)]
                wqTr = [wrp.tile([128, HD], f32r, name=f"wqTr{c}") for c in range(NK)]
                wkTr = [wrp.tile([128, HD], f32r, name=f"wkTr{c}") for c in range(NK)]
                wvTr = [wrp.tile([128, HD], f32r, name=f"wvTr{c}") for c in range(NK)]

                for c in range(NK):
                    sw = lda.tile([128, HD], f32, tag="wstage")
                    nc.sync.dma_start(sw[:], wq_d[c * 128 : (c + 1) * 128, :])
                    nc.vector.tensor_copy(wqTr[c][:], sw[:])
                    st = lda.tile([128, L], f32, tag="xstage")
                    nc.sync.dma_start(st[:], xT_d[c * 128 : (c + 1) * 128, :])
                    nc.scalar.copy(xTr[c][:], st[:])
                    sw = lda.tile([128, HD], f32, tag="wstage")
                    nc.sync.dma_start(sw[:], wk_d[c * 128 : (c + 1) * 128, :])
                    nc.vector.tensor_copy(wkTr[c][:], sw[:])
                    sw = lda.tile([128, HD], f32, tag="wstage")
                    nc.sync.dma_start(sw[:], wv_d[c * 128 : (c + 1) * 128, :])
                    nc.vector.tensor_copy(wvTr[c][:], sw[:])
                for j in range(2):
                    sw2 = lda.tile([128, D], f32, tag="wostage")
                    nc.sync.dma_start(sw2[:], wo_d[j * 128 : (j + 1) * 128, :])
                    nc.vector.tensor_copy(woTr[:, j, :], sw2[:])
                onesf = lda.tile([1, 128], f32, tag="onesf")
                nc.vector.memset(onesf[:], 1.0)
                nc.vector.tensor_copy(ones_l[:], onesf[:])

                # qT, kT: [256, L] as head-pair chunks [128, 2, L]
                for j in range(2):
                    for wt, dst in ((wqTr, qTr), (wkTr, kTr)):
                        for g in range(NG):
                            ps = psA.tile([128, 512], f32, tag="psqk")
                            for c in range(NK):
                                nc.tensor.matmul(
                                    ps[:],
                                    wt[c][:, j * 128 : (j + 1) * 128],
                                    xTr[c][:, g * 512 : (g + 1) * 512],
                                    start=(c == 0),
                                    stop=(c == NK - 1),
                                )
                            nc.vector.tensor_copy(dst[:, j, g * 512 : (g + 1) * 512], ps[:])

                # v: [L, 256] with a ones column per head ([.., 65h+64])
                onesv = lda.tile([128, HEADS_PER_CORE], f32, tag="onesv")
                nc.vector.memset(onesv[:], 1.0)
                for t in range(LT):
                    ps = psA.tile([128, 512], f32, tag="psqk")
                    for c in range(NK):
                        nc.tensor.matmul(
                            ps[:, 0:HD],
                            xTr[c][:, t * 128 : (t + 1) * 128],
                            wvTr[c][:],
                            start=(c == 0),
                            stop=(c == NK - 1),
                        )
                    vdst = v_sb[:, t, :].rearrange(
                        "p (h u) -> p h u", u=DH + 1
                    )
                    nc.vector.tensor_copy(
                        vdst[:, :, 0:DH],
                        ps[:, 0:HD].rearrange("p (h u) -> p h u", u=DH),
                    )
                    nc.vector.tensor_copy(
                        vdst[:, :, DH : DH + 1],
                        onesv[:].rearrange("p (h u) -> p h u", u=1),
                    )

            with tc.tile_pool(name="otp", bufs=1) as otp:
                ot_lo = otp.tile([64, 2, L], f32r)
                ot_hi = otp.tile([128, 2, L], f32r)
                # ------------- phase B: attention per head -------------
                with (
                    tc.tile_pool(name="ptp", bufs=2) as ptp,
                    tc.tile_pool(name="rsp", bufs=2) as rsp,
                    tc.tile_pool(name="bcp", bufs=4) as bcp,
                    tc.tile_pool(name="psST", bufs=2, space="PSUM") as psST,
                    tc.tile_pool(name="psPV", bufs=1, space="PSUM") as psPV,
                ):
                    for h in range(HEADS_PER_CORE):
                        hp, ho = h // 2, 64 * (h % 2)
                        pvs = [
                            psPV.tile([65, 512], f32, name=f"pv_h{h}_{g}", tag=f"pv{g}")
                            for g in range(NG)
                        ]
                        for m in range(LT):
                            c0 = 128 * m
                            w = L - c0
                            PT = ptp.tile([128, L], f32r, tag="pt")
                            nsub = (w + 1023) // 1024
                            for sub in range(nsub):
                                s0 = c0 + 1024 * sub
                                sw = min(1024, L - s0)
                                stp = psST.tile([128, 1024], f32, tag="st")
                                for nn in range((sw + 511) // 512):
                                    n0 = s0 + 512 * nn
                                    nw = min(512, s0 + sw - n0)
                                    nc.tensor.matmul(
                                        stp[:, nn * 512 : nn * 512 + nw],
                                        kTr[ho : ho + 64, hp, c0 : c0 + 128],
                                        qTr[ho : ho + 64, hp, n0 : n0 + nw],
                                        start=True,
                                        stop=True,
                                    )
                                if sub == 0:
                                    nc.vector.tensor_add(
                                        stp[:, 0:128], stp[:, 0:128], tm_t[:]
                                    )
                                nc.scalar.activation(
                                    PT[:, s0 - c0 : s0 - c0 + sw],
                                    stp[:, 0:sw],
                                    AF.Exp,
                                    scale=0.125,
                                )
                            for g in range(NG):
                                gs = 512 * g
                                if gs + 512 <= c0:
                                    continue
                                r0 = max(gs, c0)
                                last = m == min(LT - 1, 4 * g + 3)
                                nc.tensor.matmul(
                                    pvs[g][:, r0 - gs : 512],
                                    v_sb[:, m, h * 65 : h * 65 + 65],
                                    PT[:, r0 - c0 : gs + 512 - c0],
                                    start=(m == 0),
                                    stop=last,
                                )
                                if not last:
                                    continue
                                # g-block done at m=4g+3: normalize now so the
                                # chain overlaps the remaining chunks. The
                                # broadcast psum reuses the previous g's pv
                                # bank (already evacuated) instead of stealing
                                # an ST double-buffer slot.
                                rs_row = rsp.tile([1, 512], f32r, tag="rs")
                                nc.vector.tensor_copy(rs_row[:], pvs[g][64:65, :])
                                bc_ps = (psPV if g >= 1 else psST).tile(
                                    [128, 512], f32, name=f"bc_h{h}_{g}",
                                    tag=(f"pv{g - 1}" if g >= 1 else "st"),
                                )
                                nc.tensor.matmul(
                                    bc_ps[:], ones_l[:], rs_row[:],
                                    start=True, stop=True,
                                )
                                # 1/x as exp(-ln(x)): keeps every ACT op in the
                                # single natural_log_exp_and_others table set
                                ln_t = bcp.tile([128, 512], f32, tag="ln")
                                nc.scalar.activation(ln_t[:], bc_ps[:], AF.Ln)
                                bc_sb = bcp.tile([128, 512], f32, tag="bc")
                                nc.scalar.activation(
                                    bc_sb[:], ln_t[:], AF.Exp, scale=-1.0
                                )
                                dst = (
                                    ot_lo[:, hp, 512 * g : 512 * g + 512]
                                    if h % 2 == 0
                                    else ot_hi[64:128, hp, 512 * g : 512 * g + 512]
                                )
                                nc.vector.tensor_mul(
                                    dst, pvs[g][0:64, :], bc_sb[0:64, :]
                                )


                # ---------------- phase C: output projection ----------------
                with (
                    tc.tile_pool(name="outst", bufs=4) as outst,
                    tc.tile_pool(name="psC", bufs=4, space="PSUM") as psC,
                ):
                    for t in range(LT):
                        stage = outst.tile([128, D], f32, tag="ostage")
                        for n2 in range(2):
                            ps_a = psC.tile([128, 512], f32, tag="psa")
                            ps_b = psC.tile([128, 512], f32, tag="psb")
                            for j in range(2):
                                # even heads on array rows 0-63, odd heads on
                                # rows 64-127: pairs run concurrently in the
                                # PE array, draining to separate PSUM banks
                                nc.tensor.matmul(
                                    ps_a[:],
                                    ot_lo[:, j, t * 128 : (t + 1) * 128],
                                    woTr[0:64, j, n2 * 512 : (n2 + 1) * 512],
                                    start=(j == 0),
                                    stop=(j == 1),
                                )
                                nc.tensor.matmul(
                                    ps_b[:],
                                    ot_hi[64:128, j, t * 128 : (t + 1) * 128],
                                    woTr[64:128, j, n2 * 512 : (n2 + 1) * 512],
                                    start=(j == 0),
                                    stop=(j == 1),
                                )
                            nc.scalar.copy(
                                stage[:, n2 * 512 : (n2 + 1) * 512], ps_a[:]
                            )
                            nc.vector.tensor_add(
                                stage[:, n2 * 512 : (n2 + 1) * 512],
                                ps_b[:],
                                stage[:, n2 * 512 : (n2 + 1) * 512],
                            )
                        nc.sync.dma_start(
                            out_d[t * 128 : (t + 1) * 128, :], stage[:]
                        )



    _split_waits(nc)
    return nc



def _build_runner(nc):
    """Build the sharded PJRT executable once (mirrors
    bass2jax.run_bass_via_pjrt) and return a callable in_maps -> results."""
    import jax
    import numpy as _np
    from jax.sharding import Mesh, PartitionSpec
    from jax.experimental.shard_map import shard_map
    from concourse import bass2jax, mybir

    bass2jax.install_neuronx_cc_hook()
    partition_name = (
        nc.partition_id_tensor.name if nc.partition_id_tensor else None
    )
    in_names, out_names, out_avals, zero_outs = [], [], [], []
    for alloc in nc.m.functions[0].allocations:
        if not isinstance(alloc, mybir.MemoryLocationSet):
            continue
        name = alloc.memorylocations[0].name
        if alloc.kind == "ExternalInput":
            if name != partition_name:
                in_names.append(name)
        elif alloc.kind == "ExternalOutput":
            out_names.append(name)
            shape = tuple(alloc.tensor_shape)
            dtype = mybir.dt.np(alloc.dtype)
            out_avals.append(jax.core.ShapedArray(shape, dtype))
            zero_outs.append(_np.zeros(shape, dtype))
    n_params = len(in_names)
    n_outs = len(out_names)
    all_in_names = list(in_names) + list(out_names)
    if partition_name is not None:
        all_in_names.append(partition_name)
    donate = tuple(range(n_params, n_params + n_outs))

    def _body(*args):
        operands = list(args)
        if partition_name is not None:
            operands.append(bass2jax.partition_id_tensor())
        outs = bass2jax._bass_exec_p.bind(
            *operands,
            out_avals=tuple(out_avals),
            in_names=tuple(all_in_names),
            out_names=tuple(out_names),
            lowering_input_output_aliases=(),
            sim_require_finite=True,
            sim_require_nnan=True,
            nc=nc,
        )
        return tuple(outs)

    devices = jax.devices()[:N_CORES]
    mesh = Mesh(_np.asarray(devices), ("core",))
    in_specs = (PartitionSpec("core"),) * (n_params + n_outs)
    out_specs = (PartitionSpec("core"),) * n_outs
    sharded = jax.jit(
        shard_map(
            _body, mesh=mesh, in_specs=in_specs, out_specs=out_specs,
            check_rep=False,
        ),
        donate_argnums=donate,
        keep_unused=True,
    )

    def run(in_maps):
        concat_in = [
            _np.concatenate([_np.asarray(m[nm]) for m in in_maps], axis=0)
            for nm in in_names
        ]
        concat_zeros = [
            _np.zeros((N_CORES * z.shape[0], *z.shape[1:]), z.dtype)
            for z in zero_outs
        ]
        out_arrs = sharded(*concat_in, *concat_zeros)
        return [
            {
                nm: _np.asarray(out_arrs[i]).reshape(
                    N_CORES, *out_avals[i].shape
                )[c]
                for i, nm in enumerate(out_names)
            }
            for c in range(N_CORES)
        ]

    return run


def _numpy_ref(x, attn_mask, Wq, Wk, Wv, Wo):
    xb, Lb, Db = x.shape
    dh = Db // H
    x64 = x.astype(np.float64)
    q = (x64 @ Wq.T.astype(np.float64)).reshape(xb, Lb, H, dh)
    k = (x64 @ Wk.T.astype(np.float64)).reshape(xb, Lb, H, dh)
    v = (x64 @ Wv.T.astype(np.float64)).reshape(xb, Lb, H, dh)
    scores = np.einsum("blhd,bmhd->bhlm", q, k) / np.sqrt(dh)
    scores = np.where(attn_mask[None, None, :, :] == 0, -np.inf, scores)
    scores -= scores.max(axis=-1, keepdims=True)
    e = np.exp(scores)
    attn = e / e.sum(axis=-1, keepdims=True)
    out = np.einsum("bhlm,bmhd->blhd", attn, v).reshape(xb, Lb, Db)
    return (out @ Wo.T.astype(np.float64)).astype(x.dtype)


def _trimask():
    j = np.arange(128)
    return np.where(j[None, :] >= j[:, None], 0.0, -1.0e5).astype(np.float32)


def _make_in_maps(x, Wq, Wk, Wv, Wo):
    tm = _trimask()
    xT = [np.ascontiguousarray(x[b].T).astype(np.float32, copy=False) for b in range(B)]
    WqT = np.ascontiguousarray(Wq.T).astype(np.float32, copy=False)
    WkT = np.ascontiguousarray(Wk.T).astype(np.float32, copy=False)
    WvT = np.ascontiguousarray(Wv.T).astype(np.float32, copy=False)
    in_maps = []
    for c in range(N_CORES):
        b = c // 4
        s0 = HD * (c % 4)
        sel = slice(s0, s0 + HD)
        in_maps.append(
            {
                "xT": xT[b],
                "wqT": WqT[:, sel],
                "wkT": WkT[:, sel],
                "wvT": WvT[:, sel],
                "woT": np.ascontiguousarray(Wo[:, sel].T).astype(np.float32, copy=False),
                "trimask": tm,
            }
        )
    return in_maps


def kernel(x, attn_mask, Wq, Wk, Wv, Wo):
    x = np.asarray(x)
    attn_mask = np.asarray(attn_mask)
    Wq, Wk, Wv, Wo = (np.asarray(a) for a in (Wq, Wk, Wv, Wo))
    causal = x.shape == (B, L, D) and np.array_equal(
        attn_mask != 0, np.tril(np.ones((L, L), dtype=bool))
    )
    if not causal:
        return _numpy_ref(x, attn_mask, Wq, Wk, Wv, Wo)

    if "run" not in _CACHE:
        _CACHE["run"] = _build_runner(_build_program())
    in_maps = _make_in_maps(x, Wq, Wk, Wv, Wo)
    results = _CACHE["run"](in_maps)
    out = np.zeros((B, L, D), dtype=np.float32)
    for c in range(N_CORES):
        out[c // 4] += results[c]["out"]
    return out



# revision 6
# speedup vs baseline: 1.2094x; 1.2094x over previous
"""Multi-head self-attention (B=2, L=2048, D=1024, H=16, causal) on 8
Trainium2 NeuronCores.

Sharding: tensor-parallel over heads x data-parallel over batch.
Core c (0..7) handles batch b = c//4 and heads 4*(c%4) .. 4*(c%4)+3.
Each core computes partial = (softmax(qk^T/8) @ v_heads) @ Wo[:, cols]^T of
shape [L, D]; the host sums the 4 partials of each batch group.

Per-core kernel, v2 (all matmul operands bf16, PSUM accumulation fp32):
  - host supplies x^T (chunk-major packed) and pre-transposed bf16 weights;
    DMA lands directly in the matmul-input tiles -- no staging copies
  - q^T,k^T [256,L] come from PE matmuls c-chunk-outer so compute starts
    as soon as the first x^T chunk chunk lands; v [L,256] is computed
    per-L-tile with a memset ones column per head for free softmax
    denominators
  - scores are computed TRANSPOSED (S^T = k q^T per 128-row key chunk,
    causal tiles only), exp runs on ScalarE PSUM->SBUF producing P^T in
    bf16 exactly as the PV matmul consumes it; ScalarE runs nothing else
  - the softmax denominator row (from the v ones column) is inverted on
    VectorE, broadcast across partitions with a K=1 ones matmul (f32r
    bitcast), and the normalize multiply lands the attention output in a
    head-pair-stacked [128, L] layout
  - the output projection therefore contracts full 128-partition chunks
    (2 matmuls per 512 output cols) and is DMA'd to DRAM straight from
    PSUM
  - attention is software-pipelined one step ahead (emit S of step s+1
    before PV of step s) across head boundaries so the PE never waits on
    ScalarE exp latency
"""

import numpy as np

B, L, D, H = 2, 2048, 1024, 16
DH = D // H  # 64
N_CORES = 8
HEADS_PER_CORE = 4
HD = HEADS_PER_CORE * DH  # 256 head dims per core
NK = D // 128  # 8 contraction chunks
LT = L // 128  # 16 L tiles
NG = L // 512  # 4 column groups

_CACHE = {}


# ---------------------------------------------------------------------------
# walrus compat: this compiler build accepts at most ONE sync-wait command
# per instruction, while TileContext attaches one wait per producer proc.
# Hoist surplus waits onto same-engine NOPs inserted just before the
# offending instruction (identical AND semantics).
# ---------------------------------------------------------------------------
def _split_waits(nc):
    import bass_rust
    import concourse.mybir as mybir

    for fn in nc.m.functions:
        for bb in fn.blocks:
            insts = list(bb.instructions)
            out = []
            changed = False
            for inst in insts:
                si = inst.sync_info
                waits = list(si.on_wait) if si is not None and si.on_wait else []
                if len(waits) > 1:
                    changed = True
                    for w in waits[:-1]:
                        out.append(
                            mybir.InstNoOp(
                                name=nc.get_next_instruction_name(),
                                engine=inst.engine,
                                bass_nofuse=True,
                                sync_info=bass_rust.SyncInfo(
                                    on_wait=[w], on_update=[]
                                ),
                            )
                        )
                    inst.sync_info = bass_rust.SyncInfo(
                        on_wait=[waits[-1]], on_update=list(si.on_update or [])
                    )
                out.append(inst)
            if changed:
                try:
                    bb.instructions = out
                except Exception:
                    bb.instructions.clear()
                    bb.instructions.extend(out)


def _build_program():
    import concourse.bass as bass
    import concourse.mybir as mybir
    import concourse.tile as tile

    f32 = mybir.dt.float32
    f32r = mybir.dt.float32r
    bf16 = mybir.dt.bfloat16
    AF = mybir.ActivationFunctionType

    nc = bass.Bass("TRN2", target_bir_lowering=False, debug=False)
    # host-packed layouts (see _make_in_maps):
    #   xT    [128, NK*L]    bf16   [p, c*L + l] = x[l, c*128+p]
    #   wqkv  [128, NK*768]  bf16   [p, c*768 + s*256 + i] = Ws[c*128+p, i]
    #                               (s = 0/1/2 for q/k/v; Ws = W[sel,:].T)
    #   wo    [128, 2*D]     bf16   [p, j*D + i] = Wo[:, sel].T[j*128+p, i]
    #   trimask [128, 128]   f32    0 lower-tri / -1e5 strictly-upper
    xT_d = nc.dram_tensor("xT", [128, NK * L], bf16, kind="ExternalInput")
    wqkv_d = nc.dram_tensor("wqkv", [128, NK * 3 * HD], bf16, kind="ExternalInput")
    wo_d = nc.dram_tensor("wo", [128, 2 * D], bf16, kind="ExternalInput")
    tm_d = nc.dram_tensor("trimask", [128, 128], f32, kind="ExternalInput")
    out_d = nc.dram_tensor("out", [L, D], bf16, kind="ExternalOutput")

    with tile.TileContext(nc, pool_alloc_mode="queue") as tc:
        with tc.tile_pool(name="persist", bufs=1) as persist:
            xTr = persist.tile([128, NK, L], bf16)
            wqkv = persist.tile([128, NK, 3 * HD], bf16)
            woTr = persist.tile([128, 2, D], bf16)
            qTr = persist.tile([128, 2, L], bf16)
            kTr = persist.tile([128, 2, L], bf16)
            # per pair j: LT tiles of [65 x 2] (64 head dims + ones col)
            v_sb = persist.tile([128, 2, LT, 2 * (DH + 1)], bf16)
            tm_t = persist.tile([128, 128], f32)
            ones_f = persist.tile([1, 64], f32)
            ot = persist.tile([128, 2, L], bf16)

            nc.sync.dma_start(wqkv[:], wqkv_d[:].rearrange("p (c i) -> p c i", c=NK))
            nc.sync.dma_start(tm_t[:], tm_d[:])
            for c in range(NK):
                nc.sync.dma_start(xTr[:, c, :], xT_d[:, c * L : (c + 1) * L])
            nc.sync.dma_start(woTr[:], wo_d[:].rearrange("p (j i) -> p j i", j=2))
            nc.vector.memset(ones_f[:], 1.0)
            nc.vector.memset(
                v_sb[:].rearrange("p j t (h u) -> p j t h u", u=DH + 1)[
                    :, :, :, :, DH : DH + 1
                ],
                1.0,
            )

            # ---------------- phase A: projections ----------------
            with tc.tile_pool(name="psA", bufs=1, space="PSUM") as psA:
                # q/k per pair, c-chunk outer so compute overlaps the x DMA
                for j in range(2):
                    qps = [
                        psA.tile([128, 512], f32, name=f"qp{j}_{g}", tag=f"pA{2 * g}")
                        for g in range(NG)
                    ]
                    kps = [
                        psA.tile([128, 512], f32, name=f"kp{j}_{g}", tag=f"pA{2 * g + 1}")
                        for g in range(NG)
                    ]
                    for c in range(NK):
                        wq_c = wqkv[:, c, 0 * HD + j * 128 : 0 * HD + j * 128 + 128]
                        wk_c = wqkv[:, c, 1 * HD + j * 128 : 1 * HD + j * 128 + 128]
                        for g in range(NG):
                            nc.tensor.matmul(
                                qps[g][:],
                                wq_c,
                                xTr[:, c, g * 512 : (g + 1) * 512],
                                start=(c == 0),
                                stop=(c == NK - 1),
                            )
                            nc.tensor.matmul(
                                kps[g][:],
                                wk_c,
                                xTr[:, c, g * 512 : (g + 1) * 512],
                                start=(c == 0),
                                stop=(c == NK - 1),
                            )
                    for g in range(NG):
                        nc.vector.tensor_copy(
                            qTr[:, j, g * 512 : (g + 1) * 512], qps[g][:]
                        )
                        nc.vector.tensor_copy(
                            kTr[:, j, g * 512 : (g + 1) * 512], kps[g][:]
                        )

                # v: per L-tile, all 4 heads at once [128, 256]
                for t in range(LT):
                    vps = psA.tile([128, 512], f32, tag=f"pA{t % 8}", name=f"vp{t}")
                    for c in range(NK):
                        nc.tensor.matmul(
                            vps[:, 0:HD],
                            xTr[:, c, t * 128 : (t + 1) * 128],
                            wqkv[:, c, 2 * HD : 3 * HD],
                            start=(c == 0),
                            stop=(c == NK - 1),
                        )
                    # psum cols (j h u), u=64 -> v_sb[:, j, t, 65h+u]
                    nc.vector.tensor_copy(
                        v_sb[:, :, t, :].rearrange("p j (h u) -> p j h u", u=DH + 1)[
                            :, :, :, 0:DH
                        ],
                        vps[:, 0:HD].rearrange("p (j h u) -> p j h u", j=2, u=DH),
                    )

            # ------------- phase B: attention, software-pipelined -------------
            with (
                tc.tile_pool(name="ptp", bufs=3) as ptp,
                tc.tile_pool(name="rsp", bufs=2) as rsp,
                tc.tile_pool(name="psST", bufs=2, space="PSUM") as psST,
                tc.tile_pool(name="psPV", bufs=1, space="PSUM") as psPV,
            ):
                pvs = {}  # (h, g) -> psum tile
                pts = {}  # (h, m) -> PT tile

                def emit_S(h, m):
                    hp, ho = h // 2, 64 * (h % 2)
                    c0 = 128 * m
                    w = L - c0
                    PT = ptp.tile([128, L], bf16, tag="pt", name=f"pt{h}_{m}")
                    pts[(h, m)] = PT
                    for sub in range((w + 1023) // 1024):
                        s0 = c0 + 1024 * sub
                        sw = min(1024, L - s0)
                        stp = psST.tile([128, 1024], f32, tag="st", name=f"st{h}_{m}_{sub}")
                        for nn in range((sw + 511) // 512):
                            n0 = s0 + 512 * nn
                            nw = min(512, s0 + sw - n0)
                            nc.tensor.matmul(
                                stp[:, nn * 512 : nn * 512 + nw],
                                kTr[ho : ho + 64, hp, c0 : c0 + 128],
                                qTr[ho : ho + 64, hp, n0 : n0 + nw],
                                start=True,
                                stop=True,
                            )
                        if sub == 0:
                            nc.vector.tensor_add(
                                stp[:, 0:128], stp[:, 0:128], tm_t[:]
                            )
                        nc.scalar.activation(
                            PT[:, s0 - c0 : s0 - c0 + sw],
                            stp[:, 0:sw],
                            AF.Exp,
                            scale=0.125,
                        )

                def emit_PV(h, m):
                    hp, par = h // 2, h % 2
                    c0 = 128 * m
                    PT = pts.pop((h, m))
                    for g in range(NG):
                        gs = 512 * g
                        if gs + 512 <= c0:
                            continue
                        if m == 0:
                            pvs[(h, g)] = psPV.tile(
                                [65, 512], f32, name=f"pv_h{h}_{g}", tag=f"pv{g}"
                            )
                        pv = pvs[(h, g)]
                        r0 = max(gs, c0)
                        last = m == min(LT - 1, 4 * g + 3)
                        nc.tensor.matmul(
                            pv[:, r0 - gs : 512],
                            v_sb[:, hp, m, 65 * par : 65 * par + 65],
                            PT[:, r0 - c0 : gs + 512 - c0],
                            start=(m == 0),
                            stop=last,
                        )
                        if not last:
                            continue
                        # group g done: normalize. 1/denom on VectorE, then a
                        # K=1 ones matmul broadcasts it across 64 partitions
                        # (f32r bitcast keeps the 512-wide matmul at full
                        # rate); the multiply writes the pair-stacked ot.
                        rs_row = rsp.tile([1, 512], f32, tag="rs")
                        nc.vector.reciprocal(rs_row[:], pv[64:65, :])
                        bc_ps = (psPV if g >= 1 else psST).tile(
                            [64, 512], f32, name=f"bc_h{h}_{g}",
                            tag=(f"pv{g - 1}" if g >= 1 else "st"),
                        )
                        nc.tensor.matmul(
                            bc_ps[:],
                            ones_f[:].bitcast(f32r),
                            rs_row[:].bitcast(f32r),
                            start=True,
                            stop=True,
                        )
                        nc.vector.tensor_mul(
                            ot[64 * par : 64 * par + 64, hp, gs : gs + 512],
                            pv[0:64, :],
                            bc_ps[:],
                        )
                        del pvs[(h, g)]

                steps = [(h, m) for h in range(HEADS_PER_CORE) for m in range(LT)]
                for s in range(len(steps) + 1):
                    if s < len(steps):
                        emit_S(*steps[s])
                    if s > 0:
                        emit_PV(*steps[s - 1])

            # ---------------- phase C: output projection ----------------
            # ScalarE (idle after attention) evacuates PSUM as bf16; the
            # halved output bytes also halve the DMA drain.
            with (
                tc.tile_pool(name="outst", bufs=3) as outst,
                tc.tile_pool(name="psC", bufs=2, space="PSUM") as psC,
            ):
                for t in range(LT):
                    ps = psC.tile([128, D], f32, tag="pc")
                    for n2 in range(2):
                        for j in range(2):
                            nc.tensor.matmul(
                                ps[:, n2 * 512 : (n2 + 1) * 512],
                                ot[:, j, t * 128 : (t + 1) * 128],
                                woTr[:, j, n2 * 512 : (n2 + 1) * 512],
                                start=(j == 0),
                                stop=(j == 1),
                            )
                    stage = outst.tile([128, D], bf16, tag="ostage")
                    nc.scalar.copy(stage[:], ps[:])
                    nc.sync.dma_start(out_d[t * 128 : (t + 1) * 128, :], stage[:])

    _split_waits(nc)
    return nc


def _build_runner(nc):
    """Build the sharded PJRT executable once (mirrors
    bass2jax.run_bass_via_pjrt) and return a callable in_maps -> results."""
    import jax
    import numpy as _np
    from jax.sharding import Mesh, PartitionSpec
    from jax.experimental.shard_map import shard_map
    from concourse import bass2jax, mybir

    bass2jax.install_neuronx_cc_hook()
    partition_name = (
        nc.partition_id_tensor.name if nc.partition_id_tensor else None
    )
    in_names, out_names, out_avals, zero_outs = [], [], [], []
    for alloc in nc.m.functions[0].allocations:
        if not isinstance(alloc, mybir.MemoryLocationSet):
            continue
        name = alloc.memorylocations[0].name
        if alloc.kind == "ExternalInput":
            if name != partition_name:
                in_names.append(name)
        elif alloc.kind == "ExternalOutput":
            out_names.append(name)
            shape = tuple(alloc.tensor_shape)
            dtype = mybir.dt.np(alloc.dtype)
            out_avals.append(jax.core.ShapedArray(shape, dtype))
            zero_outs.append(_np.zeros(shape, dtype))
    n_params = len(in_names)
    n_outs = len(out_names)
    all_in_names = list(in_names) + list(out_names)
    if partition_name is not None:
        all_in_names.append(partition_name)
    donate = tuple(range(n_params, n_params + n_outs))

    def _body(*args):
        operands = list(args)
        if partition_name is not None:
            operands.append(bass2jax.partition_id_tensor())
        outs = bass2jax._bass_exec_p.bind(
            *operands,
            out_avals=tuple(out_avals),
            in_names=tuple(all_in_names),
            out_names=tuple(out_names),
            lowering_input_output_aliases=(),
            sim_require_finite=True,
            sim_require_nnan=True,
            nc=nc,
        )
        return tuple(outs)

    devices = jax.devices()[:N_CORES]
    mesh = Mesh(_np.asarray(devices), ("core",))
    in_specs = (PartitionSpec("core"),) * (n_params + n_outs)
    out_specs = (PartitionSpec("core"),) * n_outs
    sharded = jax.jit(
        shard_map(
            _body, mesh=mesh, in_specs=in_specs, out_specs=out_specs,
            check_rep=False,
        ),
        donate_argnums=donate,
        keep_unused=True,
    )

    def run(in_maps):
        concat_in = [
            _np.concatenate([_np.asarray(m[nm]) for m in in_maps], axis=0)
            for nm in in_names
        ]
        concat_zeros = [
            _np.zeros((N_CORES * z.shape[0], *z.shape[1:]), z.dtype)
            for z in zero_outs
        ]
        out_arrs = sharded(*concat_in, *concat_zeros)
        return [
            {
                nm: _np.asarray(out_arrs[i]).reshape(
                    N_CORES, *out_avals[i].shape
                )[c]
                for i, nm in enumerate(out_names)
            }
            for c in range(N_CORES)
        ]

    return run


def _numpy_ref(x, attn_mask, Wq, Wk, Wv, Wo):
    xb, Lb, Db = x.shape
    dh = Db // H
    x64 = x.astype(np.float64)
    q = (x64 @ Wq.T.astype(np.float64)).reshape(xb, Lb, H, dh)
    k = (x64 @ Wk.T.astype(np.float64)).reshape(xb, Lb, H, dh)
    v = (x64 @ Wv.T.astype(np.float64)).reshape(xb, Lb, H, dh)
    scores = np.einsum("blhd,bmhd->bhlm", q, k) / np.sqrt(dh)
    scores = np.where(attn_mask[None, None, :, :] == 0, -np.inf, scores)
    scores -= scores.max(axis=-1, keepdims=True)
    e = np.exp(scores)
    attn = e / e.sum(axis=-1, keepdims=True)
    out = np.einsum("bhlm,bmhd->blhd", attn, v).reshape(xb, Lb, Db)
    return (out @ Wo.T.astype(np.float64)).astype(x.dtype)


def _trimask():
    j = np.arange(128)
    return np.where(j[None, :] >= j[:, None], 0.0, -1.0e5).astype(np.float32)


def _make_in_maps(x, Wq, Wk, Wv, Wo):
    import ml_dtypes

    bf16 = ml_dtypes.bfloat16
    tm = _trimask()
    # xT packed [128, NK*L]: [p, c*L + l] = x[b, l, c*128 + p]
    xTp = [
        np.ascontiguousarray(
            x[b].T.reshape(NK, 128, L).transpose(1, 0, 2).reshape(128, NK * L)
        ).astype(bf16)
        for b in range(B)
    ]
    in_maps = []
    for core in range(N_CORES):
        b = core // 4
        s0 = HD * (core % 4)
        sel = slice(s0, s0 + HD)
        # Ws = W[sel, :].T  -> [D, HD]; pack [p, c*768 + s*256 + i]
        ws = np.stack(
            [Wq[sel, :].T, Wk[sel, :].T, Wv[sel, :].T], axis=0
        )  # [3, D, HD]
        wqkv = np.ascontiguousarray(
            ws.reshape(3, NK, 128, HD).transpose(2, 1, 0, 3).reshape(128, NK * 3 * HD)
        ).astype(bf16)
        # wo packed [p, j*D + i] = Wo[:, sel].T[j*128+p, i]
        woT = Wo[:, sel].T  # [HD, D]
        wo = np.ascontiguousarray(
            woT.reshape(2, 128, D).transpose(1, 0, 2).reshape(128, 2 * D)
        ).astype(bf16)
        in_maps.append(
            {"xT": xTp[b], "wqkv": wqkv, "wo": wo, "trimask": tm}
        )
    return in_maps


def kernel(x, attn_mask, Wq, Wk, Wv, Wo):
    x = np.asarray(x)
    attn_mask = np.asarray(attn_mask)
    Wq, Wk, Wv, Wo = (np.asarray(a) for a in (Wq, Wk, Wv, Wo))
    causal = x.shape == (B, L, D) and np.array_equal(
        attn_mask != 0, np.tril(np.ones((L, L), dtype=bool))
    )
    if not causal:
        return _numpy_ref(x, attn_mask, Wq, Wk, Wv, Wo)

    if "run" not in _CACHE:
        _CACHE["run"] = _build_runner(_build_program())
    in_maps = _make_in_maps(x, Wq, Wk, Wv, Wo)
    results = _CACHE["run"](in_maps)
    out = np.zeros((B, L, D), dtype=np.float32)
    for c in range(N_CORES):
        out[c // 4] += results[c]["out"].astype(np.float32)
    return out


# revision 14
# speedup vs baseline: 1.4059x; 1.1625x over previous
"""Multi-head self-attention (B=2, L=2048, D=1024, H=16, causal) on 8
Trainium2 NeuronCores.

Sharding: tensor-parallel over heads x data-parallel over batch.
Core c (0..7) handles batch b = c//4 and heads 4*(c%4) .. 4*(c%4)+3.
Each core computes partial = (softmax(qk^T/8) @ v_heads) @ Wo[:, cols]^T of
shape [L, D]; the host sums the 4 partials of each batch group.

Per-core kernel, v2 (all matmul operands bf16, PSUM accumulation fp32):
  - host supplies x^T (chunk-major packed) and pre-transposed bf16 weights;
    DMA lands directly in the matmul-input tiles -- no staging copies
  - q^T,k^T [256,L] come from PE matmuls c-chunk-outer so compute starts
    as soon as the first x^T chunk chunk lands; v [L,256] is computed
    per-L-tile with a memset ones column per head for free softmax
    denominators
  - scores are computed TRANSPOSED (S^T = k q^T per 128-row key chunk,
    causal tiles only), exp runs on ScalarE PSUM->SBUF producing P^T in
    bf16 exactly as the PV matmul consumes it; ScalarE runs nothing else
  - the softmax denominator row (from the v ones column) is inverted on
    VectorE, broadcast across partitions with a K=1 ones matmul (f32r
    bitcast), and the normalize multiply lands the attention output in a
    head-pair-stacked [128, L] layout
  - the output projection therefore contracts full 128-partition chunks
    (2 matmuls per 512 output cols) and is DMA'd to DRAM straight from
    PSUM
  - attention is software-pipelined one step ahead (emit S of step s+1
    before PV of step s) across head boundaries so the PE never waits on
    ScalarE exp latency
"""

import numpy as np

B, L, D, H = 2, 2048, 1024, 16
DH = D // H  # 64
N_CORES = 8
HEADS_PER_CORE = 4
HD = HEADS_PER_CORE * DH  # 256 head dims per core
NK = D // 128  # 8 contraction chunks
LT = L // 128  # 16 L tiles
NG = L // 512  # 4 column groups

_CACHE = {}


# ---------------------------------------------------------------------------
# walrus compat: this compiler build accepts at most ONE sync-wait command
# per instruction, while TileContext attaches one wait per producer proc.
# Hoist surplus waits onto same-engine NOPs inserted just before the
# offending instruction (identical AND semantics).
# ---------------------------------------------------------------------------
def _split_waits(nc):
    import bass_rust
    import concourse.mybir as mybir

    for fn in nc.m.functions:
        for bb in fn.blocks:
            insts = list(bb.instructions)
            out = []
            changed = False
            for inst in insts:
                si = inst.sync_info
                waits = list(si.on_wait) if si is not None and si.on_wait else []
                if len(waits) > 1:
                    changed = True
                    for w in waits[:-1]:
                        out.append(
                            mybir.InstNoOp(
                                name=nc.get_next_instruction_name(),
                                engine=inst.engine,
                                bass_nofuse=True,
                                sync_info=bass_rust.SyncInfo(
                                    on_wait=[w], on_update=[]
                                ),
                            )
                        )
                    inst.sync_info = bass_rust.SyncInfo(
                        on_wait=[waits[-1]], on_update=list(si.on_update or [])
                    )
                out.append(inst)
            if changed:
                try:
                    bb.instructions = out
                except Exception:
                    bb.instructions.clear()
                    bb.instructions.extend(out)


def _build_program():
    import concourse.bass as bass
    import concourse.mybir as mybir
    import concourse.tile as tile

    f32 = mybir.dt.float32
    f32r = mybir.dt.float32r
    bf16 = mybir.dt.bfloat16
    AF = mybir.ActivationFunctionType

    nc = bass.Bass("TRN2", target_bir_lowering=False, debug=False)
    # host-packed layouts (see _make_in_maps):
    #   xT    [128, NK*L]    bf16   [p, c*L + l] = x[l, c*128+p]
    #   wqkv  [128, NK*768]  bf16   [p, c*768 + s*256 + i] = Ws[c*128+p, i]
    #                               (s = 0/1/2 for q/k/v; Ws = W[sel,:].T)
    #   wo    [128, 2*D]     bf16   [p, j*D + i] = Wo[:, sel].T[j*128+p, i]
    #   trimask [128, 128]   bf16   0 lower-tri / -1e5 strictly-upper
    #   ident [128, 128]     bf16   identity (mask-accumulate matmul lhsT)
    xT_d = nc.dram_tensor("xT", [128, NK * L], bf16, kind="ExternalInput")
    wqkv_d = nc.dram_tensor("wqkv", [128, NK * 3 * HD], bf16, kind="ExternalInput")
    wo_d = nc.dram_tensor("wo", [128, 2 * D], bf16, kind="ExternalInput")
    tm_d = nc.dram_tensor("trimask", [128, 128], bf16, kind="ExternalInput")
    id_d = nc.dram_tensor("ident", [128, 128], bf16, kind="ExternalInput")
    out_d = nc.dram_tensor("out", [L, D], bf16, kind="ExternalOutput")

    with tile.TileContext(nc, pool_alloc_mode="queue") as tc:
        with tc.tile_pool(name="persist", bufs=1) as persist:
            xTr = persist.tile([128, NK, L], bf16)
            wqkv = persist.tile([128, NK, 3 * HD], bf16)
            woTr = persist.tile([128, 2, D], bf16)
            qTr = persist.tile([128, 2, L], bf16)
            kTr = persist.tile([128, 2, L], bf16)
            # per pair j: LT tiles of [65 x 2] (64 head dims + ones col)
            v_sb = persist.tile([128, 2, LT, 2 * (DH + 1)], bf16)
            tm_t = persist.tile([128, 128], bf16)
            ones_f = persist.tile([1, 64], f32)
            ot = persist.tile([128, 2, L], bf16)

            id_t = persist.tile([128, 128], bf16)

            nc.sync.dma_start(tm_t[:], tm_d[:])
            nc.sync.dma_start(id_t[:], id_d[:])
            # interleave weight/x chunks so projection round c can start as
            # soon as its two chunks land
            for c in range(NK):
                nc.sync.dma_start(
                    wqkv[:, c, :], wqkv_d[:, c * 3 * HD : (c + 1) * 3 * HD]
                )
                nc.sync.dma_start(xTr[:, c, :], xT_d[:, c * L : (c + 1) * L])
            nc.sync.dma_start(woTr[:], wo_d[:].rearrange("p (j i) -> p j i", j=2))
            nc.vector.memset(ones_f[:], 1.0)
            nc.vector.memset(
                v_sb[:].rearrange("p j t (h u) -> p j t h u", u=DH + 1)[
                    :, :, :, :, DH : DH + 1
                ],
                1.0,
            )

            # ---------------- phase A: pair-0 projections ----------------
            # q/k c-chunk outer so compute starts once chunk 0 lands; v after
            # (needs every chunk anyway).
            with tc.tile_pool(name="psA", bufs=1, space="PSUM") as psA:
                qps = [
                    psA.tile([128, 512], f32, name=f"qp0_{g}", tag=f"pA{2 * g}")
                    for g in range(NG)
                ]
                kps = [
                    psA.tile([128, 512], f32, name=f"kp0_{g}", tag=f"pA{2 * g + 1}")
                    for g in range(NG)
                ]
                for c in range(NK):
                    wq_c = wqkv[:, c, 0:128]
                    wk_c = wqkv[:, c, HD : HD + 128]
                    for g in range(NG):
                        nc.tensor.matmul(
                            qps[g][:], wq_c, xTr[:, c, g * 512 : (g + 1) * 512],
                            start=(c == 0), stop=(c == NK - 1),
                        )
                        nc.tensor.matmul(
                            kps[g][:], wk_c, xTr[:, c, g * 512 : (g + 1) * 512],
                            start=(c == 0), stop=(c == NK - 1),
                        )
                for g in range(NG):
                    nc.vector.tensor_copy(qTr[:, 0, g * 512 : (g + 1) * 512], qps[g][:])
                    nc.vector.tensor_copy(kTr[:, 0, g * 512 : (g + 1) * 512], kps[g][:])
                for t in range(LT):
                    vps = psA.tile([128, 512], f32, tag=f"pA{t % 8}", name=f"vp0_{t}")
                    for c in range(NK):
                        nc.tensor.matmul(
                            vps[:, 0:128],
                            xTr[:, c, t * 128 : (t + 1) * 128],
                            wqkv[:, c, 2 * HD : 2 * HD + 128],
                            start=(c == 0), stop=(c == NK - 1),
                        )
                    nc.vector.tensor_copy(
                        v_sb[:, 0, t, :].rearrange("p (h u) -> p h u", u=DH + 1)[
                            :, :, 0:DH
                        ],
                        vps[:, 0:128].rearrange("p (h u) -> p h u", u=DH),
                    )

            # ------- phase B: attention with interleaved filler work -------
            # The attention windows are ScalarE-exp paced; pair-1 projections
            # and the output projection are pumped into the PE stream as
            # "filler units" so the PE never idles on exp latency.
            with (
                tc.tile_pool(name="ptp", bufs=3) as ptp,
                tc.tile_pool(name="rsp", bufs=2) as rsp,
                tc.tile_pool(name="outst", bufs=3) as outst,
                tc.tile_pool(name="psST", bufs=2, space="PSUM") as psST,
                tc.tile_pool(name="psPV", bufs=1, space="PSUM") as psPV,
                tc.tile_pool(name="psF", bufs=1, space="PSUM") as psF,
            ):
                fidx = [0]

                def ftile(name):
                    i = fidx[0] = fidx[0] + 1
                    return psF.tile([128, 512], f32, name=f"{name}_{i}", tag=f"f{i % 2}")

                def mk_qk_unit(which, g):
                    def emit():
                        ps = ftile(f"{which}1_{g}")
                        w0 = (0 if which == "q" else HD) + 128
                        for c in range(NK):
                            nc.tensor.matmul(
                                ps[:], wqkv[:, c, w0 : w0 + 128],
                                xTr[:, c, g * 512 : (g + 1) * 512],
                                start=(c == 0), stop=(c == NK - 1),
                            )
                        dst = qTr if which == "q" else kTr
                        nc.vector.tensor_copy(dst[:, 1, g * 512 : (g + 1) * 512], ps[:])
                    return emit

                def mk_v_unit(t):
                    def emit():
                        ps = ftile(f"v1_{t}")
                        for c in range(NK):
                            nc.tensor.matmul(
                                ps[:, 0:128],
                                xTr[:, c, t * 128 : (t + 1) * 128],
                                wqkv[:, c, 2 * HD + 128 : 3 * HD],
                                start=(c == 0), stop=(c == NK - 1),
                            )
                        nc.vector.tensor_copy(
                            v_sb[:, 1, t, :].rearrange("p (h u) -> p h u", u=DH + 1)[
                                :, :, 0:DH
                            ],
                            ps[:, 0:128].rearrange("p (h u) -> p h u", u=DH),
                        )
                    return emit

                stages = {}

                def mk_c_unit(t, n2):
                    def emit():
                        ps = ftile(f"c{t}_{n2}")
                        for j in range(2):
                            nc.tensor.matmul(
                                ps[:],
                                ot[:, j, t * 128 : (t + 1) * 128],
                                woTr[:, j, n2 * 512 : (n2 + 1) * 512],
                                start=(j == 0), stop=(j == 1),
                            )
                        if n2 == 0:
                            stages[t] = outst.tile(
                                [128, D], bf16, tag="ostage", name=f"ostage{t}"
                            )
                        nc.scalar.copy(
                            stages[t][:, n2 * 512 : (n2 + 1) * 512], ps[:]
                        )
                        if n2 == 1:
                            nc.sync.dma_start(
                                out_d[t * 128 : (t + 1) * 128, :], stages.pop(t)[:]
                            )
                    return emit

                filler = []  # list of (kind, emit_fn), consumed front-first
                for g in range(NG):
                    filler.append(("qk", mk_qk_unit("q", g)))
                    filler.append(("qk", mk_qk_unit("k", g)))
                for t in range(LT):
                    filler.append(("v", mk_v_unit(t)))
                n_emitted = {"qk": 0, "v": 0, "c": 0}

                def pump(n):
                    for _ in range(n):
                        if not filler:
                            return
                        kind, fn = filler.pop(0)
                        fn()
                        n_emitted[kind] += 1

                def pump_while(cond):
                    while filler and cond():
                        kind, fn = filler.pop(0)
                        fn()
                        n_emitted[kind] += 1

                pvs = {}  # (h, g) -> psum tile
                pts = {}  # (h, m) -> PT tile

                def emit_S(h, m):
                    hp, ho = h // 2, 64 * (h % 2)
                    c0 = 128 * m
                    w = L - c0
                    PT = ptp.tile([128, L], bf16, tag="pt", name=f"pt{h}_{m}")
                    pts[(h, m)] = PT
                    for sub in range((w + 511) // 512):
                        s0 = c0 + 512 * sub
                        sw = min(512, L - s0)
                        stp = psST.tile(
                            [128, 512], f32, tag="st", name=f"st{h}_{m}_{sub}"
                        )
                        if sub == 0:
                            # causal diagonal: accumulate the mask into PSUM
                            # with an identity matmul -- keeps the whole score
                            # chain on PE (no cross-engine mask add)
                            nc.tensor.matmul(
                                stp[:, 0:128],
                                kTr[ho : ho + 64, hp, c0 : c0 + 128],
                                qTr[ho : ho + 64, hp, c0 : c0 + 128],
                                start=True, stop=False,
                            )
                            nc.tensor.matmul(
                                stp[:, 0:128], id_t[:], tm_t[:],
                                start=False, stop=True,
                            )
                            if sw > 128:
                                nc.tensor.matmul(
                                    stp[:, 128:sw],
                                    kTr[ho : ho + 64, hp, c0 : c0 + 128],
                                    qTr[ho : ho + 64, hp, s0 + 128 : s0 + sw],
                                    start=True, stop=True,
                                )
                        else:
                            nc.tensor.matmul(
                                stp[:, 0:sw],
                                kTr[ho : ho + 64, hp, c0 : c0 + 128],
                                qTr[ho : ho + 64, hp, s0 : s0 + sw],
                                start=True, stop=True,
                            )
                        nc.scalar.activation(
                            PT[:, 512 * sub : 512 * sub + sw],
                            stp[:, 0:sw],
                            AF.Exp,
                            scale=0.125,
                        )

                def emit_PV(h, m):
                    hp, par = h // 2, h % 2
                    c0 = 128 * m
                    PT = pts.pop((h, m))
                    for g in range(NG):
                        gs = 512 * g
                        if gs + 512 <= c0:
                            continue
                        if m == 0:
                            pvs[(h, g)] = psPV.tile(
                                [65, 512], f32, name=f"pv_h{h}_{g}", tag=f"pv{g}"
                            )
                        pv = pvs[(h, g)]
                        r0 = max(gs, c0)
                        last = m == min(LT - 1, 4 * g + 3)
                        nc.tensor.matmul(
                            pv[:, r0 - gs : 512],
                            v_sb[:, hp, m, 65 * par : 65 * par + 65],
                            PT[:, r0 - c0 : gs + 512 - c0],
                            start=(m == 0),
                            stop=last,
                        )
                        if not last:
                            continue
                        # group g done: normalize. 1/denom on VectorE, then a
                        # K=1 ones matmul broadcasts it across 64 partitions
                        # (f32r bitcast keeps the 512-wide matmul at full
                        # rate); the multiply writes the pair-stacked ot.
                        rs_row = rsp.tile([1, 512], f32, tag="rs")
                        nc.vector.reciprocal(rs_row[:], pv[64:65, :])
                        bc_ps = (psPV if g >= 1 else psST).tile(
                            [64, 512], f32, name=f"bc_h{h}_{g}",
                            tag=(f"pv{g - 1}" if g >= 1 else "st"),
                        )
                        nc.tensor.matmul(
                            bc_ps[:],
                            ones_f[:].bitcast(f32r),
                            rs_row[:].bitcast(f32r),
                            start=True,
                            stop=True,
                        )
                        nc.vector.tensor_mul(
                            ot[64 * par : 64 * par + 64, hp, gs : gs + 512],
                            pv[0:64, :],
                            bc_ps[:],
                        )
                        del pvs[(h, g)]
                        if h == HEADS_PER_CORE - 1:
                            # all heads done for query group g: release the
                            # output projection for its four L-tiles
                            for t in range(4 * g, 4 * g + 4):
                                for n2 in range(2):
                                    filler.append(("c", mk_c_unit(t, n2)))

                steps = [(h, m) for h in range(HEADS_PER_CORE) for m in range(LT)]
                for s in range(len(steps) + 1):
                    if s < len(steps):
                        h, m = steps[s]
                        if (h, m) == (2, 0):
                            # heads 2/3 read pair-1 q/k: flush those units
                            pump_while(lambda: n_emitted["qk"] < 2 * NG)
                        emit_S(h, m)
                    pump(2 if s >= 3 * LT else 1)
                    if s > 0:
                        h, m = steps[s - 1]
                        if h == 2:
                            # PV of head 2 chunk m reads pair-1 v tile m
                            pump_while(lambda: n_emitted["v"] <= min(m + 1, LT - 1))
                        emit_PV(h, m)
                pump(len(filler))

    _split_waits(nc)
    return nc


def _build_runner(nc):
    """Build the sharded PJRT executable once (mirrors
    bass2jax.run_bass_via_pjrt) and return a callable in_maps -> results."""
    import jax
    import numpy as _np
    from jax.sharding import Mesh, PartitionSpec
    from jax.experimental.shard_map import shard_map
    from concourse import bass2jax, mybir

    bass2jax.install_neuronx_cc_hook()
    partition_name = (
        nc.partition_id_tensor.name if nc.partition_id_tensor else None
    )
    in_names, out_names, out_avals, zero_outs = [], [], [], []
    for alloc in nc.m.functions[0].allocations:
        if not isinstance(alloc, mybir.MemoryLocationSet):
            continue
        name = alloc.memorylocations[0].name
        if alloc.kind == "ExternalInput":
            if name != partition_name:
                in_names.append(name)
        elif alloc.kind == "ExternalOutput":
            out_names.append(name)
            shape = tuple(alloc.tensor_shape)
            dtype = mybir.dt.np(alloc.dtype)
            out_avals.append(jax.core.ShapedArray(shape, dtype))
            zero_outs.append(_np.zeros(shape, dtype))
    n_params = len(in_names)
    n_outs = len(out_names)
    all_in_names = list(in_names) + list(out_names)
    if partition_name is not None:
        all_in_names.append(partition_name)
    donate = tuple(range(n_params, n_params + n_outs))

    def _body(*args):
        operands = list(args)
        if partition_name is not None:
            operands.append(bass2jax.partition_id_tensor())
        outs = bass2jax._bass_exec_p.bind(
            *operands,
            out_avals=tuple(out_avals),
            in_names=tuple(all_in_names),
            out_names=tuple(out_names),
            lowering_input_output_aliases=(),
            sim_require_finite=True,
            sim_require_nnan=True,
            nc=nc,
        )
        return tuple(outs)

    devices = jax.devices()[:N_CORES]
    mesh = Mesh(_np.asarray(devices), ("core",))
    in_specs = (PartitionSpec("core"),) * (n_params + n_outs)
    out_specs = (PartitionSpec("core"),) * n_outs
    sharded = jax.jit(
        shard_map(
            _body, mesh=mesh, in_specs=in_specs, out_specs=out_specs,
            check_rep=False,
        ),
        donate_argnums=donate,
        keep_unused=True,
    )

    def run(in_maps):
        concat_in = [
            _np.concatenate([_np.asarray(m[nm]) for m in in_maps], axis=0)
            for nm in in_names
        ]
        concat_zeros = [
            _np.zeros((N_CORES * z.shape[0], *z.shape[1:]), z.dtype)
            for z in zero_outs
        ]
        out_arrs = sharded(*concat_in, *concat_zeros)
        return [
            {
                nm: _np.asarray(out_arrs[i]).reshape(
                    N_CORES, *out_avals[i].shape
                )[c]
                for i, nm in enumerate(out_names)
            }
            for c in range(N_CORES)
        ]

    return run


def _numpy_ref(x, attn_mask, Wq, Wk, Wv, Wo):
    xb, Lb, Db = x.shape
    dh = Db // H
    x64 = x.astype(np.float64)
    q = (x64 @ Wq.T.astype(np.float64)).reshape(xb, Lb, H, dh)
    k = (x64 @ Wk.T.astype(np.float64)).reshape(xb, Lb, H, dh)
    v = (x64 @ Wv.T.astype(np.float64)).reshape(xb, Lb, H, dh)
    scores = np.einsum("blhd,bmhd->bhlm", q, k) / np.sqrt(dh)
    scores = np.where(attn_mask[None, None, :, :] == 0, -np.inf, scores)
    scores -= scores.max(axis=-1, keepdims=True)
    e = np.exp(scores)
    attn = e / e.sum(axis=-1, keepdims=True)
    out = np.einsum("bhlm,bmhd->blhd", attn, v).reshape(xb, Lb, Db)
    return (out @ Wo.T.astype(np.float64)).astype(x.dtype)


def _trimask():
    j = np.arange(128)
    return np.where(j[None, :] >= j[:, None], 0.0, -1.0e5).astype(np.float32)


def _eye128():
    return np.eye(128, dtype=np.float32)


def _make_in_maps(x, Wq, Wk, Wv, Wo):
    import ml_dtypes

    bf16 = ml_dtypes.bfloat16
    tm = _trimask().astype(bf16)
    ident = _eye128().astype(bf16)
    # xT packed [128, NK*L]: [p, c*L + l] = x[b, l, c*128 + p]
    xTp = [
        np.ascontiguousarray(
            x[b].T.reshape(NK, 128, L).transpose(1, 0, 2).reshape(128, NK * L)
        ).astype(bf16)
        for b in range(B)
    ]
    in_maps = []
    for core in range(N_CORES):
        b = core // 4
        s0 = HD * (core % 4)
        sel = slice(s0, s0 + HD)
        # Ws = W[sel, :].T  -> [D, HD]; pack [p, c*768 + s*256 + i]
        ws = np.stack(
            [Wq[sel, :].T, Wk[sel, :].T, Wv[sel, :].T], axis=0
        )  # [3, D, HD]
        wqkv = np.ascontiguousarray(
            ws.reshape(3, NK, 128, HD).transpose(2, 1, 0, 3).reshape(128, NK * 3 * HD)
        ).astype(bf16)
        # wo packed [p, j*D + i] = Wo[:, sel].T[j*128+p, i]
        woT = Wo[:, sel].T  # [HD, D]
        wo = np.ascontiguousarray(
            woT.reshape(2, 128, D).transpose(1, 0, 2).reshape(128, 2 * D)
        ).astype(bf16)
        in_maps.append(
            {"xT": xTp[b], "wqkv": wqkv, "wo": wo, "trimask": tm, "ident": ident}
        )
    return in_maps


def kernel(x, attn_mask, Wq, Wk, Wv, Wo):
    x = np.asarray(x)
    attn_mask = np.asarray(attn_mask)
    Wq, Wk, Wv, Wo = (np.asarray(a) for a in (Wq, Wk, Wv, Wo))
    causal = x.shape == (B, L, D) and np.array_equal(
        attn_mask != 0, np.tril(np.ones((L, L), dtype=bool))
    )
    if not causal:
        return _numpy_ref(x, attn_mask, Wq, Wk, Wv, Wo)

    if "run" not in _CACHE:
        _CACHE["run"] = _build_runner(_build_program())
    in_maps = _make_in_maps(x, Wq, Wk, Wv, Wo)
    results = _CACHE["run"](in_maps)
    out = np.zeros((B, L, D), dtype=np.float32)
    for c in range(N_CORES):
        out[c // 4] += results[c]["out"].astype(np.float32)
    return out


# revision 20
# speedup vs baseline: 1.4410x; 1.0250x over previous
"""Multi-head self-attention (B=2, L=2048, D=1024, H=16, causal) on 8
Trainium2 NeuronCores.

Sharding: tensor-parallel over heads x data-parallel over batch.
Core c (0..7) handles batch b = c//4 and heads 4*(c%4) .. 4*(c%4)+3.
Each core computes partial = (softmax(qk^T/8) @ v_heads) @ Wo[:, cols]^T of
shape [L, D]; the host sums the 4 partials of each batch group.

Per-core kernel, v2 (all matmul operands bf16, PSUM accumulation fp32):
  - host supplies x^T (chunk-major packed) and pre-transposed bf16 weights;
    DMA lands directly in the matmul-input tiles -- no staging copies
  - q^T,k^T [256,L] come from PE matmuls c-chunk-outer so compute starts
    as soon as the first x^T chunk chunk lands; v [L,256] is computed
    per-L-tile with a memset ones column per head for free softmax
    denominators
  - scores are computed TRANSPOSED (S^T = k q^T per 128-row key chunk,
    causal tiles only), exp runs on ScalarE PSUM->SBUF producing P^T in
    bf16 exactly as the PV matmul consumes it; ScalarE runs nothing else
  - the softmax denominator row (from the v ones column) is inverted on
    VectorE, broadcast across partitions with a K=1 ones matmul (f32r
    bitcast), and the normalize multiply lands the attention output in a
    head-pair-stacked [128, L] layout
  - the output projection therefore contracts full 128-partition chunks
    (2 matmuls per 512 output cols) and is DMA'd to DRAM straight from
    PSUM
  - attention is software-pipelined one step ahead (emit S of step s+1
    before PV of step s) across head boundaries so the PE never waits on
    ScalarE exp latency
"""

import numpy as np

B, L, D, H = 2, 2048, 1024, 16
DH = D // H  # 64
N_CORES = 8
HEADS_PER_CORE = 4
HD = HEADS_PER_CORE * DH  # 256 head dims per core
NK = D // 128  # 8 contraction chunks
LT = L // 128  # 16 L tiles
NG = L // 512  # 4 column groups

_CACHE = {}


# ---------------------------------------------------------------------------
# walrus compat: this compiler build accepts at most ONE sync-wait command
# per instruction, while TileContext attaches one wait per producer proc.
# Hoist surplus waits onto same-engine NOPs inserted just before the
# offending instruction (identical AND semantics).
# ---------------------------------------------------------------------------
def _split_waits(nc):
    import bass_rust
    import concourse.mybir as mybir

    for fn in nc.m.functions:
        for bb in fn.blocks:
            insts = list(bb.instructions)
            out = []
            changed = False
            for inst in insts:
                si = inst.sync_info
                waits = list(si.on_wait) if si is not None and si.on_wait else []
                if len(waits) > 1:
                    changed = True
                    for w in waits[:-1]:
                        out.append(
                            mybir.InstNoOp(
                                name=nc.get_next_instruction_name(),
                                engine=inst.engine,
                                bass_nofuse=True,
                                sync_info=bass_rust.SyncInfo(
                                    on_wait=[w], on_update=[]
                                ),
                            )
                        )
                    inst.sync_info = bass_rust.SyncInfo(
                        on_wait=[waits[-1]], on_update=list(si.on_update or [])
                    )
                out.append(inst)
            if changed:
                try:
                    bb.instructions = out
                except Exception:
                    bb.instructions.clear()
                    bb.instructions.extend(out)


def _build_program():
    import concourse.bass as bass
    import concourse.mybir as mybir
    import concourse.tile as tile

    f32 = mybir.dt.float32
    f32r = mybir.dt.float32r
    bf16 = mybir.dt.bfloat16
    AF = mybir.ActivationFunctionType

    nc = bass.Bass("TRN2", target_bir_lowering=False, debug=False)
    # host-packed layouts (see _make_in_maps):
    #   xT    [128, NK*L]    bf16   [p, c*L + l] = x[l, c*128+p]
    #   wqkv  [128, NK*768]  bf16   [p, c*768 + s*256 + i] = Ws[c*128+p, i]
    #                               (s = 0/1/2 for q/k/v; Ws = W[sel,:].T)
    #   wo    [128, 2*D]     bf16   [p, j*D + i] = Wo[:, sel].T[j*128+p, i]
    #   trimask [128, 128]   bf16   0 lower-tri / -1e5 strictly-upper
    #   ident [128, 128]     bf16   identity (mask-accumulate matmul lhsT)
    xT_d = nc.dram_tensor("xT", [128, NK * L], bf16, kind="ExternalInput")
    wqkv_d = nc.dram_tensor("wqkv", [128, NK * 3 * HD], bf16, kind="ExternalInput")
    wo_d = nc.dram_tensor("wo", [128, 2 * D], bf16, kind="ExternalInput")
    tm_d = nc.dram_tensor("trimask", [128, 128], bf16, kind="ExternalInput")
    id_d = nc.dram_tensor("ident", [128, 128], bf16, kind="ExternalInput")
    out_d = nc.dram_tensor("out", [L, D], bf16, kind="ExternalOutput")

    with tile.TileContext(nc, pool_alloc_mode="queue") as tc:
        with tc.tile_pool(name="persist", bufs=1) as persist:
            xTr = persist.tile([128, NK, L], bf16)
            wqkv = persist.tile([128, NK, 3 * HD], bf16)
            woTr = persist.tile([128, 2, D], bf16)
            qTr = persist.tile([128, 2, L], bf16)
            kTr = persist.tile([128, 2, L], bf16)
            # per pair j: LT tiles of [65 x 2] (64 head dims + ones col)
            v_sb = persist.tile([128, 2, LT, 2 * (DH + 1)], bf16)
            tm_t = persist.tile([128, 128], bf16)
            ones_f = persist.tile([1, 64], f32)
            ot = persist.tile([128, 2, L], bf16)

            id_t = persist.tile([128, 128], bf16)

            # interleave weight/x chunks so projection round c can start as
            # soon as its two chunks land; round-0 chunks lead everything
            for c in range(NK):
                nc.sync.dma_start(
                    wqkv[:, c, :], wqkv_d[:, c * 3 * HD : (c + 1) * 3 * HD]
                )
                nc.sync.dma_start(xTr[:, c, :], xT_d[:, c * L : (c + 1) * L])
                if c == 0:
                    nc.sync.dma_start(tm_t[:], tm_d[:])
                    nc.sync.dma_start(id_t[:], id_d[:])
            nc.sync.dma_start(woTr[:], wo_d[:].rearrange("p (j i) -> p j i", j=2))
            nc.vector.memset(ones_f[:], 1.0)
            nc.vector.memset(
                v_sb[:].rearrange("p j t (h u) -> p j t h u", u=DH + 1)[
                    :, :, :, :, DH : DH + 1
                ],
                1.0,
            )

            # ---------------- phase A: pair-0 projections ----------------
            # q/k c-chunk outer so compute starts once chunk 0 lands; v after
            # (needs every chunk anyway).
            with tc.tile_pool(name="psA", bufs=1, space="PSUM") as psA:
                qps = [
                    psA.tile([128, 512], f32, name=f"qp0_{g}", tag=f"pA{2 * g}")
                    for g in range(NG)
                ]
                kps = [
                    psA.tile([128, 512], f32, name=f"kp0_{g}", tag=f"pA{2 * g + 1}")
                    for g in range(NG)
                ]
                for c in range(NK):
                    wq_c = wqkv[:, c, 0:128]
                    wk_c = wqkv[:, c, HD : HD + 128]
                    for g in range(NG):
                        nc.tensor.matmul(
                            qps[g][:], wq_c, xTr[:, c, g * 512 : (g + 1) * 512],
                            start=(c == 0), stop=(c == NK - 1),
                        )
                        nc.tensor.matmul(
                            kps[g][:], wk_c, xTr[:, c, g * 512 : (g + 1) * 512],
                            start=(c == 0), stop=(c == NK - 1),
                        )
                for g in range(NG):
                    nc.vector.tensor_copy(qTr[:, 0, g * 512 : (g + 1) * 512], qps[g][:])
                    nc.vector.tensor_copy(kTr[:, 0, g * 512 : (g + 1) * 512], kps[g][:])
                for t in range(LT):
                    vps = psA.tile([128, 512], f32, tag=f"pA{t % 8}", name=f"vp0_{t}")
                    for c in range(NK):
                        nc.tensor.matmul(
                            vps[:, 0:128],
                            xTr[:, c, t * 128 : (t + 1) * 128],
                            wqkv[:, c, 2 * HD : 2 * HD + 128],
                            start=(c == 0), stop=(c == NK - 1),
                        )
                    nc.vector.tensor_copy(
                        v_sb[:, 0, t, :].rearrange("p (h u) -> p h u", u=DH + 1)[
                            :, :, 0:DH
                        ],
                        vps[:, 0:128].rearrange("p (h u) -> p h u", u=DH),
                    )

            # ------- phase B: attention with interleaved filler work -------
            # The attention windows are ScalarE-exp paced; pair-1 projections
            # and the output projection are pumped into the PE stream as
            # "filler units" so the PE never idles on exp latency.
            with (
                tc.tile_pool(name="ptp", bufs=3) as ptp,
                tc.tile_pool(name="rsp", bufs=2) as rsp,
                tc.tile_pool(name="outst", bufs=3) as outst,
                tc.tile_pool(name="psST", bufs=2, space="PSUM") as psST,
                tc.tile_pool(name="psPV", bufs=1, space="PSUM") as psPV,
                tc.tile_pool(name="psF", bufs=1, space="PSUM") as psF,
            ):
                fidx = [0]

                def ftile(name):
                    i = fidx[0] = fidx[0] + 1
                    return psF.tile([128, 512], f32, name=f"{name}_{i}", tag=f"f{i % 2}")

                def mk_qk_unit(which, g):
                    def emit():
                        ps = ftile(f"{which}1_{g}")
                        w0 = (0 if which == "q" else HD) + 128
                        for c in range(NK):
                            nc.tensor.matmul(
                                ps[:], wqkv[:, c, w0 : w0 + 128],
                                xTr[:, c, g * 512 : (g + 1) * 512],
                                start=(c == 0), stop=(c == NK - 1),
                            )
                        dst = qTr if which == "q" else kTr
                        nc.vector.tensor_copy(dst[:, 1, g * 512 : (g + 1) * 512], ps[:])
                    return emit

                def mk_v_unit(t):
                    def emit():
                        ps = ftile(f"v1_{t}")
                        for c in range(NK):
                            nc.tensor.matmul(
                                ps[:, 0:128],
                                xTr[:, c, t * 128 : (t + 1) * 128],
                                wqkv[:, c, 2 * HD + 128 : 3 * HD],
                                start=(c == 0), stop=(c == NK - 1),
                            )
                        nc.vector.tensor_copy(
                            v_sb[:, 1, t, :].rearrange("p (h u) -> p h u", u=DH + 1)[
                                :, :, 0:DH
                            ],
                            ps[:, 0:128].rearrange("p (h u) -> p h u", u=DH),
                        )
                    return emit

                stages = {}

                def mk_c_unit(t, n2):
                    def emit():
                        ps = ftile(f"c{t}_{n2}")
                        for j in range(2):
                            nc.tensor.matmul(
                                ps[:],
                                ot[:, j, t * 128 : (t + 1) * 128],
                                woTr[:, j, n2 * 512 : (n2 + 1) * 512],
                                start=(j == 0), stop=(j == 1),
                            )
                        # evacuation alternates ScalarE/VectorE so the drain
                        # after the last attention group is not single-engine
                        if n2 == 0:
                            stages[t] = outst.tile(
                                [128, D], bf16, tag="ostage", name=f"ostage{t}"
                            )
                            nc.scalar.copy(stages[t][:, 0:512], ps[:])
                        else:
                            nc.vector.tensor_copy(stages[t][:, 512:D], ps[:])
                        if n2 == 1:
                            nc.sync.dma_start(
                                out_d[t * 128 : (t + 1) * 128, :], stages.pop(t)[:]
                            )
                    return emit

                filler = []  # list of (kind, emit_fn), consumed front-first
                for g in range(NG):
                    filler.append(("qk", mk_qk_unit("q", g)))
                    filler.append(("qk", mk_qk_unit("k", g)))
                for t in range(LT):
                    filler.append(("v", mk_v_unit(t)))
                n_emitted = {"qk": 0, "v": 0, "c": 0}

                def pump(n):
                    for _ in range(n):
                        if not filler:
                            return
                        kind, fn = filler.pop(0)
                        fn()
                        n_emitted[kind] += 1

                def pump_while(cond):
                    while filler and cond():
                        kind, fn = filler.pop(0)
                        fn()
                        n_emitted[kind] += 1

                pvs = {}  # (h, g) -> psum tile
                pts = {}  # (h, m) -> PT tile

                def emit_S(h, m):
                    hp, ho = h // 2, 64 * (h % 2)
                    c0 = 128 * m
                    w = L - c0
                    PT = ptp.tile([128, L], bf16, tag="pt", name=f"pt{h}_{m}")
                    pts[(h, m)] = PT
                    for sub in range((w + 511) // 512):
                        s0 = c0 + 512 * sub
                        sw = min(512, L - s0)
                        stp = psST.tile(
                            [128, 512], f32, tag="st", name=f"st{h}_{m}_{sub}"
                        )
                        if sub == 0:
                            # causal diagonal: accumulate the mask into PSUM
                            # with an identity matmul -- keeps the whole score
                            # chain on PE (no cross-engine mask add)
                            nc.tensor.matmul(
                                stp[:, 0:128],
                                kTr[ho : ho + 64, hp, c0 : c0 + 128],
                                qTr[ho : ho + 64, hp, c0 : c0 + 128],
                                start=True, stop=False,
                            )
                            nc.tensor.matmul(
                                stp[:, 0:128], id_t[:], tm_t[:],
                                start=False, stop=True,
                            )
                            if sw > 128:
                                nc.tensor.matmul(
                                    stp[:, 128:sw],
                                    kTr[ho : ho + 64, hp, c0 : c0 + 128],
                                    qTr[ho : ho + 64, hp, s0 + 128 : s0 + sw],
                                    start=True, stop=True,
                                )
                        else:
                            nc.tensor.matmul(
                                stp[:, 0:sw],
                                kTr[ho : ho + 64, hp, c0 : c0 + 128],
                                qTr[ho : ho + 64, hp, s0 : s0 + sw],
                                start=True, stop=True,
                            )
                        nc.scalar.activation(
                            PT[:, 512 * sub : 512 * sub + sw],
                            stp[:, 0:sw],
                            AF.Exp,
                            scale=0.125,
                        )

                def emit_PV(h, m):
                    hp, par = h // 2, h % 2
                    c0 = 128 * m
                    PT = pts.pop((h, m))
                    for g in range(NG):
                        gs = 512 * g
                        if gs + 512 <= c0:
                            continue
                        if m == 0:
                            pvs[(h, g)] = psPV.tile(
                                [65, 512], f32, name=f"pv_h{h}_{g}", tag=f"pv{g}"
                            )
                        pv = pvs[(h, g)]
                        r0 = max(gs, c0)
                        last = m == min(LT - 1, 4 * g + 3)
                        nc.tensor.matmul(
                            pv[:, r0 - gs : 512],
                            v_sb[:, hp, m, 65 * par : 65 * par + 65],
                            PT[:, r0 - c0 : gs + 512 - c0],
                            start=(m == 0),
                            stop=last,
                        )
                        if not last:
                            continue
                        # group g done: normalize. 1/denom on VectorE, then a
                        # K=1 ones matmul broadcasts it across 64 partitions
                        # (f32r bitcast keeps the 512-wide matmul at full
                        # rate); the multiply writes the pair-stacked ot.
                        rs_row = rsp.tile([1, 512], f32, tag="rs")
                        nc.vector.reciprocal(rs_row[:], pv[64:65, :])
                        bc_ps = ftile(f"bc_h{h}_{g}")
                        nc.tensor.matmul(
                            bc_ps[0:64, :],
                            ones_f[:].bitcast(f32r),
                            rs_row[:].bitcast(f32r),
                            start=True,
                            stop=True,
                        )
                        nc.vector.tensor_mul(
                            ot[64 * par : 64 * par + 64, hp, gs : gs + 512],
                            pv[0:64, :],
                            bc_ps[0:64, :],
                        )
                        del pvs[(h, g)]
                        if h == HEADS_PER_CORE - 1:
                            # all heads done for query group g: release the
                            # output projection for its four L-tiles
                            for t in range(4 * g, 4 * g + 4):
                                for n2 in range(2):
                                    filler.append(("c", mk_c_unit(t, n2)))

                # two-step software pipeline: PV consumes step s-2, so the
                # S -> exp -> PV chain latency is fully hidden behind a whole
                # step of PE work plus the pumped filler units
                steps = [(h, m) for h in range(HEADS_PER_CORE) for m in range(LT)]
                for s in range(len(steps) + 2):
                    if s < len(steps):
                        h, m = steps[s]
                        if (h, m) == (2, 0):
                            # heads 2/3 read pair-1 q/k: flush those units
                            pump_while(lambda: n_emitted["qk"] < 2 * NG)
                        emit_S(h, m)
                    pump(2 if s >= 3 * LT else 1)
                    if s >= 2:
                        h, m = steps[s - 2]
                        if h == 2:
                            # PV of head 2 chunk m reads pair-1 v tile m
                            pump_while(lambda: n_emitted["v"] <= min(m + 1, LT - 1))
                        emit_PV(h, m)
                pump(len(filler))

    _split_waits(nc)
    return nc


def _build_runner(nc):
    """Build the sharded PJRT executable once (mirrors
    bass2jax.run_bass_via_pjrt) and return a callable in_maps -> results."""
    import jax
    import numpy as _np
    from jax.sharding import Mesh, PartitionSpec
    from jax.experimental.shard_map import shard_map
    from concourse import bass2jax, mybir

    bass2jax.install_neuronx_cc_hook()
    partition_name = (
        nc.partition_id_tensor.name if nc.partition_id_tensor else None
    )
    in_names, out_names, out_avals, zero_outs = [], [], [], []
    for alloc in nc.m.functions[0].allocations:
        if not isinstance(alloc, mybir.MemoryLocationSet):
            continue
        name = alloc.memorylocations[0].name
        if alloc.kind == "ExternalInput":
            if name != partition_name:
                in_names.append(name)
        elif alloc.kind == "ExternalOutput":
            out_names.append(name)
            shape = tuple(alloc.tensor_shape)
            dtype = mybir.dt.np(alloc.dtype)
            out_avals.append(jax.core.ShapedArray(shape, dtype))
            zero_outs.append(_np.zeros(shape, dtype))
    n_params = len(in_names)
    n_outs = len(out_names)
    all_in_names = list(in_names) + list(out_names)
    if partition_name is not None:
        all_in_names.append(partition_name)
    donate = tuple(range(n_params, n_params + n_outs))

    def _body(*args):
        operands = list(args)
        if partition_name is not None:
            operands.append(bass2jax.partition_id_tensor())
        outs = bass2jax._bass_exec_p.bind(
            *operands,
            out_avals=tuple(out_avals),
            in_names=tuple(all_in_names),
            out_names=tuple(out_names),
            lowering_input_output_aliases=(),
            sim_require_finite=True,
            sim_require_nnan=True,
            nc=nc,
        )
        return tuple(outs)

    devices = jax.devices()[:N_CORES]
    mesh = Mesh(_np.asarray(devices), ("core",))
    in_specs = (PartitionSpec("core"),) * (n_params + n_outs)
    out_specs = (PartitionSpec("core"),) * n_outs
    sharded = jax.jit(
        shard_map(
            _body, mesh=mesh, in_specs=in_specs, out_specs=out_specs,
            check_rep=False,
        ),
        donate_argnums=donate,
        keep_unused=True,
    )

    def run(in_maps):
        concat_in = [
            _np.concatenate([_np.asarray(m[nm]) for m in in_maps], axis=0)
            for nm in in_names
        ]
        concat_zeros = [
            _np.zeros((N_CORES * z.shape[0], *z.shape[1:]), z.dtype)
            for z in zero_outs
        ]
        out_arrs = sharded(*concat_in, *concat_zeros)
        return [
            {
                nm: _np.asarray(out_arrs[i]).reshape(
                    N_CORES, *out_avals[i].shape
                )[c]
                for i, nm in enumerate(out_names)
            }
            for c in range(N_CORES)
        ]

    return run


def _numpy_ref(x, attn_mask, Wq, Wk, Wv, Wo):
    xb, Lb, Db = x.shape
    dh = Db // H
    x64 = x.astype(np.float64)
    q = (x64 @ Wq.T.astype(np.float64)).reshape(xb, Lb, H, dh)
    k = (x64 @ Wk.T.astype(np.float64)).reshape(xb, Lb, H, dh)
    v = (x64 @ Wv.T.astype(np.float64)).reshape(xb, Lb, H, dh)
    scores = np.einsum("blhd,bmhd->bhlm", q, k) / np.sqrt(dh)
    scores = np.where(attn_mask[None, None, :, :] == 0, -np.inf, scores)
    scores -= scores.max(axis=-1, keepdims=True)
    e = np.exp(scores)
    attn = e / e.sum(axis=-1, keepdims=True)
    out = np.einsum("bhlm,bmhd->blhd", attn, v).reshape(xb, Lb, Db)
    return (out @ Wo.T.astype(np.float64)).astype(x.dtype)


def _trimask():
    j = np.arange(128)
    return np.where(j[None, :] >= j[:, None], 0.0, -1.0e5).astype(np.float32)


def _eye128():
    return np.eye(128, dtype=np.float32)


def _make_in_maps(x, Wq, Wk, Wv, Wo):
    import ml_dtypes

    bf16 = ml_dtypes.bfloat16
    tm = _trimask().astype(bf16)
    ident = _eye128().astype(bf16)
    # xT packed [128, NK*L]: [p, c*L + l] = x[b, l, c*128 + p]
    xTp = [
        np.ascontiguousarray(
            x[b].T.reshape(NK, 128, L).transpose(1, 0, 2).reshape(128, NK * L)
        ).astype(bf16)
        for b in range(B)
    ]
    in_maps = []
    for core in range(N_CORES):
        b = core // 4
        s0 = HD * (core % 4)
        sel = slice(s0, s0 + HD)
        # Ws = W[sel, :].T  -> [D, HD]; pack [p, c*768 + s*256 + i]
        ws = np.stack(
            [Wq[sel, :].T, Wk[sel, :].T, Wv[sel, :].T], axis=0
        )  # [3, D, HD]
        wqkv = np.ascontiguousarray(
            ws.reshape(3, NK, 128, HD).transpose(2, 1, 0, 3).reshape(128, NK * 3 * HD)
        ).astype(bf16)
        # wo packed [p, j*D + i] = Wo[:, sel].T[j*128+p, i]
        woT = Wo[:, sel].T  # [HD, D]
        wo = np.ascontiguousarray(
            woT.reshape(2, 128, D).transpose(1, 0, 2).reshape(128, 2 * D)
        ).astype(bf16)
        in_maps.append(
            {"xT": xTp[b], "wqkv": wqkv, "wo": wo, "trimask": tm, "ident": ident}
        )
    return in_maps


def kernel(x, attn_mask, Wq, Wk, Wv, Wo):
    x = np.asarray(x)
    attn_mask = np.asarray(attn_mask)
    Wq, Wk, Wv, Wo = (np.asarray(a) for a in (Wq, Wk, Wv, Wo))
    causal = x.shape == (B, L, D) and np.array_equal(
        attn_mask != 0, np.tril(np.ones((L, L), dtype=bool))
    )
    if not causal:
        return _numpy_ref(x, attn_mask, Wq, Wk, Wv, Wo)

    if "run" not in _CACHE:
        _CACHE["run"] = _build_runner(_build_program())
    in_maps = _make_in_maps(x, Wq, Wk, Wv, Wo)
    results = _CACHE["run"](in_maps)
    out = np.zeros((B, L, D), dtype=np.float32)
    for c in range(N_CORES):
        out[c // 4] += results[c]["out"].astype(np.float32)
    return out
